# revision 16
# baseline (speedup 1.0000x reference)
"""Trainium2 Bass kernel for nn_BertGTHead_37177236914708 (BertGT pooling head).

Full-input contract: kernel(**inputs) takes the complete (unsharded) numpy
inputs and returns the full [B, 1+G] float32 output.

Strategy (data-parallel over batch, 2 examples per NeuronCore, 8 cores):
  - Host compacts each example's valid rows (base_mask=1, ~50% of S) into a
    dense order-preserving fp16 array, zero-padded to capacity C. The device
    streams C rows/example with no mask ops (zero-padding is exact for sums
    and absorbed by the final max(.,0)).
  - Stream: chunks of (4,4,4,4,2) 128-row j-groups per example. Each chunk
    is ONE merged [128, 2*j*H] tile: ex0's half arrives on the sync HWDGE
    ring, ex1's on the scalar ring (first 3 chunks) or the GpSimd SWDGE
    queue (tail chunks). This keeps the scalar ring at 4 DMAs (HWDGE issue
    instructions block the issuing engine when its queue fills, which would
    starve the ScalarE PSUM copies), and HWDGE descriptor generation
    (~3.2us per 128-partition DMA) off the critical path.
  - text max: per-chunk halving-tree tensor_tensor max over BOTH examples
    at once (4D strided APs) into a merged [128, 2*H] fp16 accumulator on
    VectorE; fp16 PE transposes to h-partition layout at the end.
  - text sum: PE ones-matmuls accumulated in PSUM. The two examples' MMs
    are interleaved so consecutive MMs hit different PSUM banks (reuse
    distance 4 hides the accumulate-turnaround bubble), and a 12-MM warmup
    before the stream ramps the PE out of its low-clock p-state. The [1,H]
    sums are PE-transposed to h-partitions and join the fused cls dot.
  - windows: host pre-gathers AND pre-masks the 32 rows around each gap
    (index/mask-shaped prep, same class as the host-gathered centers); all
    small data (windows, centers, fp16 identity, weights) is packed into
    ONE fp16 tensor (wtpx) split across the two HWDGE rings.
  - final scores: per-partition dots, then one ones-matmul sums the 128
    h-partials for all 34 outputs in output order.
"""

import numpy as np
from contextlib import ExitStack

# ---- problem constants (hardcoded; harness runs kernel.py standalone) ----
B, S, H, G = 16, 4096, 768, 16
WIN = 15
WLEN = 2 * WIN + 1           # 31
NCORES = 8
EX = B // NCORES             # 2 examples per core
P = 128
JS = (4, 4, 4, 4, 2)         # j-groups (128-row blocks) per stream chunk
NCH = len(JS)
JTOT = sum(JS)               # 18 j-groups = C/P rows per example
C_MIN = P * JTOT             # 2304 (capacity: valid rows per example)
SCALAR_CH = 3                # ex1 chunks 0..2 ride the scalar ring, rest SWDGE
OB = 4                       # 8-row blocks per (32-row padded) window
OB_R = 8                     # rows per block
NOUT = 1 + G                 # 17 scores per example
NE = EX * G                  # 32 gaps per core

# packed fp16 aux tensor (wtpx) column layout
WX_WIN = 0                   # [0, 6144): pre-masked window slots (8 x H)
WX_CTR = OB_R * H            # [6144, 6912): centers (rows 0..NE-1)
WX_ID = WX_CTR + H           # [6912, 7040): fp16 identity
WX_AUX = WX_ID + P           # [7040, 7152): weights/scalars
WX_COLS = WX_AUX + 112

_BUILT = {}                  # capacity C -> compiled Bacc
_C = C_MIN


def _build(C):
    """Build + compile the per-core Bass program for capacity C (cached)."""
    if C in _BUILT:
        return _BUILT[C]

    import concourse.bacc as bacc
    import concourse.bass as bass
    import concourse.tile as tile
    from concourse import mybir
    from concourse.masks import make_identity

    f16 = mybir.dt.float16
    f32 = mybir.dt.float32
    AF = mybir.ActivationFunctionType
    OP = mybir.AluOpType
    AX = mybir.AxisListType

    CH = C // C_MIN  # scale factor if capacity grew (stays 1 normally)
    assert C == C_MIN * CH
    JSc = tuple(j * CH for j in JS)

    nc = bacc.Bacc("TRN2", target_bir_lowering=False, debug=False,
                   num_devices=NCORES)

    xc_d = nc.dram_tensor("xc", [EX * C, H], f16, kind="ExternalInput").ap()
    wtpx_d = nc.dram_tensor("wtpx", [P, WX_COLS], f16,
                            kind="ExternalInput").ap()
    out_d = nc.dram_tensor("out", [EX * NOUT], f32, kind="ExternalOutput").ap()

    with tile.TileContext(nc) as tc, ExitStack() as ctx:
        singles = ctx.enter_context(tc.tile_pool(name="singles", bufs=1))
        xpool = ctx.enter_context(tc.tile_pool(name="xin", bufs=1))
        accpool = ctx.enter_context(tc.tile_pool(name="acc", bufs=1))
        winpool = ctx.enter_context(tc.tile_pool(name="win", bufs=1))
        smalls = ctx.enter_context(tc.tile_pool(name="smalls", bufs=4))
        foldp = ctx.enter_context(tc.tile_pool(name="fold", bufs=2))
        pacc = ctx.enter_context(tc.tile_pool(name="pacc", bufs=2, space="PSUM"))
        pbig = ctx.enter_context(tc.tile_pool(name="pbig", bufs=1, space="PSUM"))
        pbigc = ctx.enter_context(tc.tile_pool(name="pbigc", bufs=1, space="PSUM"))
        psq = ctx.enter_context(tc.tile_pool(name="psq", bufs=1, space="PSUM"))
        pout = ctx.enter_context(tc.tile_pool(name="pout", bufs=1, space="PSUM"))

        # ---- ring DMAs. Each chunk tile holds both examples' j-groups:
        # cols [0, jH) = ex0 (sync ring), [jH, 2jH) = ex1 (scalar ring for
        # chunks 0..SCALAR_CH-1, GpSimd SWDGE for the tail) ----
        xts = {}

        def chunk_dma(ex, T):
            j = JSc[T]
            if (ex, T) not in xts and (1 - ex, T) not in xts:
                xts[(0, T)] = xts[(1, T)] = xpool.tile(
                    [P, EX * j * H], f16, name=f"xt{T}", tag=f"xt{T}")
            xt = xts[(ex, T)]
            row0 = ex * C + sum(JSc[:T]) * P
            src = bass.AP(xc_d.tensor, row0 * H, [[j * H, P], [1, j * H]])
            if ex == 0:
                eng = nc.sync
            elif T < SCALAR_CH:
                eng = nc.scalar
            else:
                eng = nc.gpsimd
            eng.dma_start(out=xt[:, ex * j * H:(ex + 1) * j * H], in_=src)

        wtpx = winpool.tile([P, WX_COLS], f16)
        chunk_dma(0, 0)
        chunk_dma(1, 0)
        nc.sync.dma_start(out=wtpx[:, 0:4 * H], in_=wtpx_d[:, 0:4 * H])
        nc.scalar.dma_start(out=wtpx[:, 4 * H:WX_COLS],
                            in_=wtpx_d[:, 4 * H:WX_COLS])
        for T in range(1, NCH):
            chunk_dma(0, T)
            chunk_dma(1, T)

        id16 = wtpx[:, WX_ID:WX_ID + P]
        aux = wtpx[:, WX_AUX:WX_AUX + 112]
        pooled_a = aux[:, 0:12]              # 2ex x 6
        cwc_a = aux[:, 12:48]                # 2ex x 18 (max | pooled | sum)
        invcnt_a = aux[:, 48:80]             # 32
        gwt_a = aux[:, 80:98]                # 18
        invtn_a = aux[:, 98:100]             # 1/num_tokens per ex

        # ---- stream compute pieces ----
        acc2 = accpool.tile([P, EX * H], f16)    # merged max accumulator
        msc = accpool.tile([P, 4 * H], f16)      # fold scratch
        pss = [pacc.tile([1, H], f32, name=f"ps{e}", tag="ps")
               for e in range(EX)]

        # PE p-state warmup: the PE idles until the first stream chunk
        # lands; ~12 dummy matmuls keep it executing so it ramps to full
        # clock before the real accumulation starts. They write the ps
        # banks, which the real accumulation group resets via start=True.
        wsc = singles.tile([P, 512], f16)
        nc.vector.memset(wsc[:], 0.0)
        ones16 = singles.tile([P, 1], f16)
        nc.vector.memset(ones16[:], 1.0)
        for i in range(12):
            nc.tensor.matmul(out=pss[i % EX][0:1, 0:512], lhsT=ones16[:],
                             rhs=wsc[:], start=True, stop=True)

        ident = singles.tile([P, P], f32)
        make_identity(nc, ident[:])
        ones = singles.tile([P, 1], f32)
        nc.vector.memset(ones[:], 1.0)
        # one-time ACT table load, after the scalar-ring DMAs are issued
        warm = singles.tile([1, 1], f32)
        nc.scalar.activation(out=warm[:], in_=ones[0:1, 0:1], func=AF.Copy)

        # rhs of the final ones-matmul, in output order:
        # col ex*17 = cls partials, cols ex*17+1+g = gap partials
        rhs34 = smalls.tile([P, EX * NOUT], f32)

        def stream_mm(T):
            # interleave examples and PSUM half-banks so consecutive MMs
            # never re-accumulate the same bank (hides the PSUM turnaround)
            j = JSc[T]
            xt = xts[(0, T)]
            for jj in range(j):
                first = (T == 0 and jj == 0)
                last = (T == NCH - 1 and jj == j - 1)
                for ex in range(EX):
                    o = (ex * j + jj) * H
                    nc.tensor.matmul(out=pss[ex][0:1, 0:512],
                                     lhsT=ones16[:],
                                     rhs=xt[:, o:o + 512],
                                     start=first, stop=last)
                for ex in range(EX):
                    o = (ex * j + jj) * H
                    nc.tensor.matmul(out=pss[ex][0:1, 512:H],
                                     lhsT=ones16[:],
                                     rhs=xt[:, o + 512:o + H],
                                     start=first, stop=last)

        def stream_max(T):
            # halving-tree max over the chunk's j-groups for BOTH examples
            # at once: view [P, (ex, n, H)], fold n -> n/2 per op
            xt = xts[(0, T)]
            n = JSc[T]
            src_t, src_off, stride = xt[:], 0, JSc[T] * H
            while n > 1:
                n //= 2
                in0 = bass.AP(src_t.tensor, src_t.offset + src_off,
                              [src_t.ap[0], [stride, EX], [1, n * H]])
                in1 = bass.AP(src_t.tensor, src_t.offset + src_off + n * H,
                              [src_t.ap[0], [stride, EX], [1, n * H]])
                if n == 1 and T == 0:
                    ot = acc2
                else:
                    ot = msc
                out = bass.AP(ot[:].tensor, ot[:].offset,
                              [ot[:].ap[0], [n * H, EX], [1, n * H]])
                nc.vector.tensor_tensor(out=out, in0=in0, in1=in1, op=OP.max)
                src_t, src_off, stride = ot[:], 0, n * H
            if T > 0:
                nc.vector.tensor_tensor(out=acc2[:], in0=acc2[:],
                                        in1=msc[:, 0:EX * H], op=OP.max)

        ws = winpool.tile([P, OB_R * H // 2], f16)
        wsF = winpool.tile([P, H], f16)
        wm = winpool.tile([P, OB_R * H // 2], f16)
        wtF = winpool.tile([P, H], f16)
        gfeat = winpool.tile([P, 3 * 6 * NE], f32)       # [cT|maxT|sumT]
        gfold = winpool.tile([P, 2 * 6 * NE], f32)

        def emit_window_a():
            # sum/max trees on VectorE fp16 (rows pre-mask-zeroed on host)
            wt_sb = wtpx[:, 0:OB_R * H]
            nc.vector.tensor_tensor(out=ws[:], in0=wt_sb[:, 0:4 * H],
                                    in1=wt_sb[:, 4 * H:8 * H], op=OP.add)
            nc.vector.tensor_tensor(out=ws[:, 0:2 * H], in0=ws[:, 0:2 * H],
                                    in1=ws[:, 2 * H:4 * H], op=OP.add)
            nc.vector.tensor_tensor(out=wsF[:], in0=ws[:, 0:H],
                                    in1=ws[:, H:2 * H], op=OP.add)
            nc.vector.tensor_tensor(out=wm[:], in0=wt_sb[:, 0:4 * H],
                                    in1=wt_sb[:, 4 * H:8 * H], op=OP.max)
            nc.vector.tensor_tensor(out=wm[:, 0:2 * H], in0=wm[:, 0:2 * H],
                                    in1=wm[:, 2 * H:4 * H], op=OP.max)
            nc.vector.tensor_tensor(out=wtF[:], in0=wm[:, 0:H],
                                    in1=wm[:, H:2 * H], op=OP.max)
            # centers (rows 0..NE-1 of the ctr block): fp16 PE transpose
            ptC = pbigc.tile([P, 6 * NE], f16)
            for c in range(6):
                nc.tensor.transpose(
                    out=ptC[:, c * NE:(c + 1) * NE],
                    in_=wtpx[0:NE, WX_CTR + c * P:WX_CTR + (c + 1) * P],
                    identity=id16[0:NE, 0:NE])
            nc.scalar.activation(out=gfeat[:, 0:6 * NE], in_=ptC[:],
                                 func=AF.Copy)

        def emit_window_b():
            # transpose to h-partition layout, copy PSUM->SBUF on ScalarE,
            # fold the 4 ob-groups with cheap strided TT ops
            def obfold(gm, dst, op):
                # gm free layout: c*128 + ob*32 + e (c in 6, ob in 4, e in 32)
                g = gm[:]
                v = [bass.AP(g.tensor, g.offset + ob * NE,
                             [g.ap[0], [P, 6], [1, NE]]) for ob in range(OB)]
                f = gfold[:]
                f01 = bass.AP(f.tensor, f.offset, [f.ap[0], [NE, 6], [1, NE]])
                f23 = bass.AP(f.tensor, f.offset + 6 * NE,
                              [f.ap[0], [NE, 6], [1, NE]])
                d = bass.AP(dst.tensor, dst.offset,
                            [dst.ap[0], [NE, 6], [1, NE]])
                nc.vector.tensor_tensor(out=f01, in0=v[0], in1=v[1], op=op)
                nc.vector.tensor_tensor(out=f23, in0=v[2], in1=v[3], op=op)
                nc.vector.tensor_tensor(out=d, in0=f01, in1=f23, op=op)

            ptM = pbig.tile([P, H], f16, tag="ptw")
            for c in range(6):
                nc.tensor.transpose(out=ptM[:, c * P:(c + 1) * P],
                                    in_=wtF[:, c * P:(c + 1) * P],
                                    identity=id16)
            gmM = winpool.tile([P, H], f32)
            nc.scalar.activation(out=gmM[:], in_=ptM[:], func=AF.Copy)
            obfold(gmM, gfeat[:, 6 * NE:12 * NE], OP.max)
            nc.vector.tensor_scalar_max(out=gfeat[:, 6 * NE:12 * NE],
                                        in0=gfeat[:, 6 * NE:12 * NE],
                                        scalar1=0.0)
            ptS = pbig.tile([P, H], f16, tag="ptw")
            for c in range(6):
                nc.tensor.transpose(out=ptS[:, c * P:(c + 1) * P],
                                    in_=wsF[:, c * P:(c + 1) * P],
                                    identity=id16)
            gmS = winpool.tile([P, H], f32)
            nc.scalar.activation(out=gmS[:], in_=ptS[:], func=AF.Copy)
            obfold(gmS, gfeat[:, 12 * NE:18 * NE], OP.add)
            # avg = sum / cnt  (per (ex,g) along free)
            icnt_b = bass.AP(invcnt_a.tensor, invcnt_a.offset,
                             [invcnt_a.ap[0], [0, 6], [1, NE]])
            gf_s = bass.AP(gfeat[:].tensor, gfeat[:].offset + 12 * NE,
                           [gfeat[:].ap[0], [NE, 6], [1, NE]])
            nc.vector.tensor_tensor(out=gf_s, in0=gf_s, in1=icnt_b,
                                    op=OP.mult)

            # combined gap dot: gfeat[p,(part,c,exg)] * W[part*H + c*128 + p]
            gw_b = bass.AP(gwt_a.tensor, gwt_a.offset,
                           [gwt_a.ap[0], [6, 3], [1, 6], [0, NE]])
            gf_v = bass.AP(gfeat[:].tensor, gfeat[:].offset,
                           [gfeat[:].ap[0], [6 * NE, 3], [NE, 6], [1, NE]])
            nc.vector.tensor_tensor(out=gf_v, in0=gf_v, in1=gw_b, op=OP.mult)
            gf_r = bass.AP(gfeat[:].tensor, gfeat[:].offset,
                           [gfeat[:].ap[0], [1, NE], [NE, 18]])
            rhs_g = bass.AP(rhs34[:].tensor, rhs34[:].offset + 1,
                            [rhs34[:].ap[0], [NOUT, EX], [1, G]])
            nc.vector.tensor_reduce(out=rhs_g, in_=gf_r, axis=AX.X,
                                    op=OP.add)

        def finalize_ex(ex):
            maxf = acc2[:, ex * H:(ex + 1) * H]
            pt = pbig.tile([P, H], f16, tag="ptw")
            for c in range(6):
                nc.tensor.transpose(out=pt[:, c * P:(c + 1) * P],
                                    in_=maxf[:, c * P:(c + 1) * P],
                                    identity=id16)
            ptsb = foldp.tile([P, H], f16)
            nc.scalar.activation(out=ptsb[:], in_=pt[:], func=AF.Copy)
            feat6 = foldp.tile([P, 6], f32)
            pt_v = ptsb[:].rearrange("p (c s) -> p c s", c=6)
            nc.vector.tensor_reduce(out=feat6[:], in_=pt_v, axis=AX.X,
                                    op=OP.max)
            # zero-padding may be absent (nv == C): floor at 0 here
            nc.vector.tensor_scalar_max(out=feat6[:], in0=feat6[:],
                                        scalar1=0.0)

            # text-sum: PSUM [1,H] -> SBUF -> PE transpose to h-partitions
            ps = pss[ex]
            pssb = foldp.tile([1, H], f32, name=f"pssb{ex}", tag="pssb")
            nc.scalar.activation(out=pssb[:], in_=ps[:], func=AF.Copy)
            ptq = psq.tile([P, 6], f32, name=f"ptq{ex}", tag="ptq")
            for c in range(6):
                nc.tensor.transpose(out=ptq[:, c:c + 1],
                                    in_=pssb[0:1, c * P:(c + 1) * P],
                                    identity=ident[0:1, 0:1])
            sum6 = foldp.tile([P, 6], f32, name=f"sum6{ex}", tag="sum6")
            nc.scalar.activation(out=sum6[:], in_=ptq[:], func=AF.Copy)
            # fold in 1/num_tokens (per-example scalar, broadcast over free)
            itn_b = bass.AP(invtn_a.tensor, invtn_a.offset + ex,
                            [invtn_a.ap[0], [0, 6]])
            nc.vector.tensor_tensor(out=sum6[:], in0=sum6[:], in1=itn_b,
                                    op=OP.mult)

            # cls partials: [text-max | pooled | text-sum/tn] . cls_W
            cprod = foldp.tile([P, 18], f32)
            nc.vector.tensor_tensor(out=cprod[:, 0:6], in0=feat6[:],
                                    in1=cwc_a[:, ex * 18:ex * 18 + 6],
                                    op=OP.mult)
            nc.vector.tensor_tensor(out=cprod[:, 6:12],
                                    in0=pooled_a[:, ex * 6:(ex + 1) * 6],
                                    in1=cwc_a[:, ex * 18 + 6:ex * 18 + 12],
                                    op=OP.mult)
            nc.vector.tensor_tensor(out=cprod[:, 12:18], in0=sum6[:],
                                    in1=cwc_a[:, ex * 18 + 12:ex * 18 + 18],
                                    op=OP.mult)
            cidx = ex * NOUT
            nc.vector.tensor_reduce(out=rhs34[:, cidx:cidx + 1],
                                    in_=cprod[:], axis=AX.X, op=OP.add)

        # ---- emission order: chunk-0 compute first (it lands first), the
        # window head once wtpx lands, window tail mid-stream, per-example
        # finalization, output ----
        stream_mm(0)
        stream_max(0)
        emit_window_a()
        for T in range(1, NCH):
            stream_mm(T)
            stream_max(T)
            if T == 2:
                emit_window_b()
        for ex in range(EX):
            finalize_ex(ex)

        # ---- final ones-matmul (sums partials over h' partitions) ----
        pscore = pout.tile([1, EX * NOUT], f32)
        nc.tensor.matmul(out=pscore[:], lhsT=ones[:], rhs=rhs34[:],
                         start=True, stop=True)
        sg = smalls.tile([1, EX * NOUT], f32)
        nc.scalar.activation(out=sg[:], in_=pscore[:], func=AF.Copy)
        nc.sync.dma_start(out=out_d[:], in_=sg[0:1, :])

    nc.compile()
    _BUILT[C] = nc
    return nc


def _prep_core(seq_c, pooled_c, bm_c, gids_c, gW, cW, C):
    """Host-side per-core input prep. seq_c [EX,S,H] f32 (view), bm_c [EX,S]
    bool, gids_c [EX,G] int, gW [3H] f32, cW [3H] f32, C = capacity."""
    f32 = np.float32
    f16 = np.float16

    xc = np.zeros((EX * C, H), dtype=f16)
    a = np.empty((EX, G), dtype=np.int64)
    b = np.empty((EX, G), dtype=np.int64)
    tn = np.empty((EX,), dtype=f32)
    for ex in range(EX):
        pos = np.flatnonzero(bm_c[ex])
        nv = len(pos)
        tn[ex] = nv
        xc[ex * C:ex * C + nv] = seq_c[ex, pos].astype(f16)
        a[ex] = np.searchsorted(pos, gids_c[ex] - WIN, side="left")
        b[ex] = np.searchsorted(pos, gids_c[ex] + WIN, side="right")

    # window partitions: p = ob*32 + ex*16 + g; each holds OB_R=8 compacted
    # rows starting at row start + ob*8 of a 32-row padded block, pre-masked
    obv = np.repeat(np.arange(OB), NE)            # [P]
    exv = np.tile(np.repeat(np.arange(EX), G), OB)
    gv = np.tile(np.arange(G), EX * OB)
    a_p = a[exv, gv]                              # [P]
    b_p = b[exv, gv]
    start = np.clip(a_p, 0, C - OB * OB_R)        # [P] padded-block start
    rows = (start + obv * OB_R)[:, None] + np.arange(OB_R)[None, :]  # [P, 8]
    wmask = ((rows >= a_p[:, None]) & (rows < b_p[:, None]))
    gath = xc[(exv * C)[:, None] + rows]          # [P, 8, H] f16

    cnt = (b - a).astype(f32)                     # [EX, G]
    with np.errstate(divide="ignore"):
        icnt = 1.0 / cnt

    exg_e = np.repeat(np.arange(EX), G)
    exg_g = np.tile(np.arange(G), EX)
    ctr = seq_c[exg_e, gids_c[exg_e, exg_g]]      # [NE, H] f32

    wtpx = np.zeros((P, WX_COLS), dtype=f16)
    wtpx[:, 0:OB_R * H] = (gath * wmask[:, :, None].astype(f16)
                           ).reshape(P, OB_R * H)
    wtpx[0:NE, WX_CTR:WX_CTR + H] = ctr.astype(f16)
    wtpx[:, WX_ID:WX_ID + P] = np.eye(P, dtype=f16)

    # aux block: [0:12] pooledr, [12:48] cwc (max|pooled|sum per ex),
    # [48:80] invcnt, [80:98] gwt, [98:100] 1/tn
    aux = np.zeros((P, 112), f32)
    cw3 = cW.reshape(3, 6, P)                     # [part, c, p]
    for ex in range(EX):
        aux[:, ex * 6:(ex + 1) * 6] = pooled_c[ex].reshape(6, P).T
        o = 12 + ex * 18
        aux[:, o:o + 6] = cw3[1].T                # text-max weights
        aux[:, o + 6:o + 12] = cw3[0].T           # pooled weights
        aux[:, o + 12:o + 18] = cw3[2].T          # text-sum weights
        aux[:, 98 + ex] = 1.0 / tn[ex]
    aux[:, 48:80] = np.broadcast_to(icnt.reshape(NE), (P, NE))
    aux[:, 80:98] = gW.reshape(3, 6, P).transpose(2, 0, 1).reshape(P, 18)
    wtpx[:, WX_AUX:WX_AUX + 112] = aux.astype(f16)

    return {"xc": xc, "wtpx": wtpx}


def _make_in_maps(sequence_output, pooled_output, token_type_ids, word_mask,
                  gap_ids, gap_W, cls_W):
    global _C
    seq = np.asarray(sequence_output, dtype=np.float32)
    pooled = np.asarray(pooled_output, dtype=np.float32)
    tti = np.asarray(token_type_ids)
    wmk = np.asarray(word_mask)
    gids = np.asarray(gap_ids).astype(np.int64)
    gW = np.asarray(gap_W, dtype=np.float32)
    cW = np.asarray(cls_W, dtype=np.float32)
    base_mask = (tti == 0) & (wmk != 0)

    max_nv = int(base_mask.sum(axis=1).max())
    C = max(C_MIN, -(-max_nv // C_MIN) * C_MIN)
    # keep the compiled capacity if it still fits (avoid rebuilds)
    if _BUILT and any(c >= C for c in _BUILT):
        C = min(c for c in _BUILT if c >= C)
    _C = C

    in_maps = []
    for c in range(NCORES):
        lo = c * EX
        in_maps.append(_prep_core(seq[lo:lo + EX], pooled[lo:lo + EX],
                                  base_mask[lo:lo + EX], gids[lo:lo + EX],
                                  gW, cW, C))
    return in_maps


def _run(in_maps, trace=False, trace_cores=None):
    from concourse import bass_utils
    nc = _build(_C)
    return bass_utils.run_bass_kernel_spmd(
        nc, in_maps, core_ids=list(range(NCORES)), trace=trace,
        trace_cores=trace_cores)


def kernel(sequence_output, pooled_output, token_type_ids, word_mask,
           gap_ids, gap_W, gap_b, cls_W, cls_b):
    in_maps = _make_in_maps(sequence_output, pooled_output, token_type_ids,
                            word_mask, gap_ids, gap_W, cls_W)
    res = _run(in_maps)
    out = np.concatenate(
        [res.results[c]["out"].reshape(EX, NOUT) for c in range(NCORES)], 0)
    out[:, 0] += float(np.asarray(cls_b))
    out[:, 1:] += float(np.asarray(gap_b))
    return out.astype(np.float32)
